# revision 20
# baseline (speedup 1.0000x reference)
"""Multi-head attention (BN-folded QKV + rel-pos bias + GELU + out-proj) on 8 TRN2 cores.

Data-parallel over batch (b=8 -> 1 batch element per core, no collectives).

All BatchNorms are eval-mode affine transforms folded into the projection
weights/biases on the host.  The additive Toeplitz position bias is folded in
multiplicatively after exp:  exp(dots + E) = exp(dots) * exp(E), with exp(E)
shipped as per-partition pre-shifted window tiles.

v2 schedule (vs v1):
 - ACT (scalar) engine is the serial floor (64 softmax exps).  Exps are
   batched 2048-wide (one ACTIVATE per two attention units) reading a
   persistent 3-slice PSUM ring via strided APs.
 - ic is the outer loop within each head pair so the attn@v accumulators
   only hold 2 PSUM banks, freeing 6 banks for the dots ring.
 - v-projection units ride the same PSUM ring, interleaved with early
   attention units; q/k projections use the attnv accumulator banks before
   attention starts.
 - Input DMA is issued in consumption order on one queue (x, wq, wk, biases,
   wv, then the exp-window table in per-pair chunks) and dummy PE matmuls
   bridge the DMA wait so the PE hits the first projection with a warm HAM
   clock.
"""

import numpy as np
import ml_dtypes

HEADS, DK, DV = 8, 32, 64
DIM, N, DIM_OUT = 256, 1024, 256
IDK, IDV = DK * HEADS, DV * HEADS  # 256, 512
SCALE = DK ** -0.5
EPS = 1e-5
B = 8
WIN = 1920  # window tile width: covers i - 128*jc in [-896, 1023]

BF16 = ml_dtypes.bfloat16


def _prep_host(x, Wq, Wk, Wv, Wo, bo, pos_emb,
               q_gamma, q_beta, q_mean, q_var,
               k_gamma, k_beta, k_mean, k_var,
               v_gamma, v_beta, v_mean, v_var,
               o_gamma, o_beta, o_mean, o_var):
    f32 = np.float32
    inv_q = (q_gamma / np.sqrt(q_var + EPS)).astype(f32)
    inv_k = (k_gamma / np.sqrt(k_var + EPS)).astype(f32)
    inv_v = (v_gamma / np.sqrt(v_var + EPS)).astype(f32)
    inv_o = (o_gamma / np.sqrt(o_var + EPS)).astype(f32)

    # q also absorbs the attention scale
    Wq_eff = (Wq * inv_q[:, None]) * SCALE
    bq = ((q_beta - q_mean * inv_q) * SCALE).astype(f32)
    Wk_eff = Wk * inv_k[:, None]
    bk = (k_beta - k_mean * inv_k).astype(f32)
    Wv_eff = Wv * inv_v[:, None]
    bv = (v_beta - v_mean * inv_v).astype(f32)
    Wo_eff = Wo * inv_o[:, None]
    bo_eff = ((bo - o_mean) * inv_o + o_beta).astype(f32)

    # lhsT layouts, pre-chunked to the exact SBUF tile shapes
    def chunk_T(w, kchunks):  # [O, C] -> [128, kchunks, O]  (WT[c, o] tiled)
        wT = np.ascontiguousarray(w.T.astype(f32))  # [C, O]
        c, o = wT.shape
        assert c == kchunks * 128
        return np.ascontiguousarray(
            wT.reshape(kchunks, 128, o).transpose(1, 0, 2)).astype(BF16)

    wqs = chunk_T(Wq_eff, 2)            # [128, 2, 256]
    wks = chunk_T(Wk_eff, 2)            # [128, 2, 256]
    wvs = chunk_T(Wv_eff, 2)            # [128, 2, 512]
    wos = chunk_T(Wo_eff, 4)            # [128, 4, 256]

    def chunk_bias(b, chunks):  # [C] -> [128, chunks]
        return np.ascontiguousarray(
            b.reshape(chunks, 128).T).astype(f32)

    bos = chunk_bias(bo_eff, 2)         # [128, 2]
    # q/k/v biases enter their projections as a ones-row matmul (K=1)
    bqrow = bq.reshape(1, IDK).astype(BF16)  # [1, 256]
    bkrow = bk.reshape(1, IDK).astype(BF16)  # [1, 256]
    bvrow = bv.reshape(1, IDV).astype(BF16)  # [1, 512]  (h-major: c = h*64+dv)

    # exp-window tiles: for j = 128*jc + p, attn[p, i] needs
    # expE[1023 + i - j] = win[p, h, (896 - 128*jc) + i]
    E = (np.asarray(pos_emb, dtype=np.float64) / SCALE)  # [N, HEADS]
    d = np.abs(np.arange(2047) - 1023)
    expE = np.exp(E[d, :])  # [2047, HEADS] float64
    idx = np.arange(WIN)[None, :] - np.arange(128)[:, None] + 127  # [128, WIN]
    win = expE[idx, :].transpose(0, 2, 1)  # [128, HEADS, WIN]
    win = np.ascontiguousarray(win).astype(BF16)

    shared = dict(wqs=wqs, wks=wks, wvs=wvs, wos=wos,
                  bqrow=bqrow, bkrow=bkrow, bos=bos, bvrow=bvrow, win=win)
    return shared


def _x_shard(x, i):
    # device consumes x as bf16 [128, 2, n] (channel chunks c = a*128 + p)
    xi = np.asarray(x[i], dtype=np.float32).reshape(2, 128, N).transpose(1, 0, 2)
    return np.ascontiguousarray(xi).astype(BF16)


def _build_nc():
    import concourse.bass as bass
    import concourse.mybir as mybir
    import concourse.tile as tile
    from concourse import bacc

    f32 = mybir.dt.float32
    bf16 = mybir.dt.bfloat16

    nc = bacc.Bacc(None, target_bir_lowering=False)

    x_ext = nc.declare_dram_parameter("x", [128, 2, N], bf16, isOutput=False)
    wqs_ext = nc.declare_dram_parameter("wqs", [128, 2, IDK], bf16, isOutput=False)
    wks_ext = nc.declare_dram_parameter("wks", [128, 2, IDK], bf16, isOutput=False)
    wvs_ext = nc.declare_dram_parameter("wvs", [128, 2, IDV], bf16, isOutput=False)
    wos_ext = nc.declare_dram_parameter("wos", [128, 4, DIM_OUT], bf16, isOutput=False)
    bqrow_ext = nc.declare_dram_parameter("bqrow", [1, IDK], bf16, isOutput=False)
    bkrow_ext = nc.declare_dram_parameter("bkrow", [1, IDK], bf16, isOutput=False)
    bos_ext = nc.declare_dram_parameter("bos", [128, 2], f32, isOutput=False)
    bvrow_ext = nc.declare_dram_parameter("bvrow", [1, IDV], bf16, isOutput=False)
    win_ext = nc.declare_dram_parameter("win", [128, HEADS, WIN], bf16, isOutput=False)
    out_ext = nc.declare_dram_parameter("out", [DIM_OUT, N], f32, isOutput=True)

    Exp = mybir.ActivationFunctionType.Exp
    Gelu = mybir.ActivationFunctionType.Gelu
    Identity = mybir.ActivationFunctionType.Identity

    AP = bass.AP

    with tile.TileContext(nc) as tc:
        with (
            tc.tile_pool(name="consts", bufs=1) as consts,
            tc.tile_pool(name="attnp", bufs=5) as attnp,
            tc.tile_pool(name="normp", bufs=4) as normp,
            tc.tile_pool(name="outp", bufs=4) as outp,
            tc.tile_pool(name="psum", bufs=1, space="PSUM") as psum,
        ):
            # ---- persistent PSUM: dots ring (6 banks) + attnv accum (2) ----
            dps = psum.tile([128, 3, 1024], f32)   # ring of 3 unit slices
            aps = psum.tile([128, 2, 512], f32)    # attnv accum / qkproj psums

            # ---- SBUF constants / intermediates ----
            xb = consts.tile([128, 2, N], bf16)
            wq = consts.tile([128, 2, IDK], bf16)
            wk = consts.tile([128, 2, IDK], bf16)
            wv = consts.tile([128, 2, IDV], bf16)
            wo = consts.tile([128, 4, DIM_OUT], bf16)
            bqr = consts.tile([1, IDK], bf16)
            bkr = consts.tile([1, IDK], bf16)
            bos = consts.tile([128, 2], f32)
            bvr = consts.tile([1, IDV], bf16)
            win = consts.tile([128, HEADS, WIN], bf16)
            ones1 = consts.tile([1, 512], bf16)

            q_sb = consts.tile([128, 2, N], bf16)   # [ (h,d) chunks, i ]
            k_sb = consts.tile([128, 2, N], bf16)   # [ (h,d) chunks, j ]
            # v columns 0:32 ones (sums 32-replicated for the DVE reciprocal),
            # 32:64 zero (dark PE cells; PSUM reads must stay quadrant-aligned
            # so out_u lives at partitions 64:128), 64:128 = v + bv.
            v_aug = consts.tile([128, 8, HEADS, 128], bf16)  # [j-part, jc, h, one|0|dv]
            g_sb = consts.tile([128, 4, N], bf16)   # gelu in/out [(h,dv) chunks, i]

            # ---- input DMA, single queue, consumption order ----
            nc.sync.dma_start(out=xb, in_=x_ext[:])
            nc.sync.dma_start(out=wq, in_=wqs_ext[:])
            nc.sync.dma_start(out=wk, in_=wks_ext[:])
            nc.sync.dma_start(out=bqr, in_=bqrow_ext[:])
            nc.sync.dma_start(out=bkr, in_=bkrow_ext[:])
            nc.sync.dma_start(out=wv, in_=wvs_ext[:])
            nc.sync.dma_start(out=bvr, in_=bvrow_ext[:])
            # win chunks: pair p needs heads 2p:2p+2; u>=768 for jc pair (0,1),
            # then 256-wide strips marching left for jc pairs (2,3),(4,5),(6,7)
            win_ranges = [(768, 1920), (512, 768), (256, 512), (0, 256)]
            win_done = {}
            for p in range(4):
                for ci, (u0, u1) in enumerate(win_ranges):
                    nc.sync.dma_start(
                        out=win[:, 2 * p:2 * p + 2, u0:u1],
                        in_=win_ext[:, 2 * p:2 * p + 2, u0:u1])
                    win_done[(p, ci)] = True
                if p == 0:
                    nc.sync.dma_start(out=wo, in_=wos_ext[:])
                    nc.sync.dma_start(out=bos, in_=bos_ext[:])

            # ---- engine warmups (no DMA deps) ----
            nc.vector.memset(ones1, 1.0)
            # ones/zero regions of v_aug on gpsimd (idle engine)
            nc.gpsimd.memset(v_aug[:, :, :, 0:32], 1.0)
            nc.gpsimd.memset(v_aug[:, :, :, 32:DV], 0.0)
            # ACT exp-table warm load
            warm = normp.tile([1, 8], f32, tag="warm", bufs=1)
            nc.vector.memset(warm, 1.0)
            nc.scalar.activation(warm, warm, Exp)
            # PE warm matmuls: keep the PE busy through the x/wq DMA wait so
            # HAM grants the 2.4GHz clock by the time real work lands.
            for _ in range(8):
                nc.tensor.matmul(aps[:, 0, :], lhsT=ones1[:, 0:128],
                                 rhs=ones1[:, 0:512], start=True, stop=True)

            # ---- q/k projections: psums on the aps ring.  The channel bias
            # rides a K=1 ones-row matmul so the evacuation is a plain
            # table-free copy on ACT (no Identity table-set thrash). ----
            def emit_qkproj(mc, w_t, b_r, dst, ic, slot):
                ps = aps[:, slot, :]
                for kc in range(2):
                    nc.tensor.matmul(
                        ps,
                        lhsT=w_t[:, kc, mc * 128:(mc + 1) * 128],
                        rhs=xb[:, kc, ic * 512:(ic + 1) * 512],
                        start=(kc == 0), stop=False)
                nc.tensor.matmul(
                    ps, lhsT=b_r[:, mc * 128:(mc + 1) * 128],
                    rhs=ones1[:, 0:512], start=False, stop=True)
                nc.scalar.copy(dst[:, mc, ic * 512:(ic + 1) * 512], ps)

            # mc0 q/k ic0+ic1 first: they gate pair-0 attention
            emit_qkproj(0, wq, bqr, q_sb, 0, 0)
            emit_qkproj(0, wk, bkr, k_sb, 0, 1)
            emit_qkproj(0, wq, bqr, q_sb, 1, 0)
            emit_qkproj(0, wk, bkr, k_sb, 1, 1)

            # ---- ring state ----
            ring = [0]  # next dps slice index

            def take_slice():
                s = ring[0]
                ring[0] = (s + 1) % 3
                return s

            # ---- v projection pseudo-units: two jc per ring slice ----
            def emit_vproj2(jc):  # jc, jc+1
                s = take_slice()
                for d in range(2):
                    ps = dps[:, s, d * 512:(d + 1) * 512]
                    for kc in range(2):
                        nc.tensor.matmul(
                            ps,
                            lhsT=xb[:, kc, (jc + d) * 128:(jc + d + 1) * 128],
                            rhs=wv[:, kc, :],
                            start=(kc == 0), stop=False)
                    nc.tensor.matmul(ps, lhsT=ones1[:, 0:128], rhs=bvr,
                                     start=False, stop=True)
                # one batched cast for both jc
                nc.vector.tensor_copy(
                    v_aug[:, jc:jc + 2, :, DV:128],
                    dps[:, s, :].rearrange("p (j h d) -> p j h d", j=2, h=HEADS))

            # ---- attention ----
            KOFF = [((h % 4) * 32, h // 4) for h in range(HEADS)]

            def emit_dots(p, ic, jc):
                s = take_slice()
                h0, h1 = 2 * p, 2 * p + 1
                koff0, kch0 = KOFF[h0]
                koff1, kch1 = KOFF[h1]
                nc.tensor.matmul(
                    dps[:, s, 0:512],
                    lhsT=k_sb[koff0:koff0 + 32, kch0, jc * 128:(jc + 1) * 128],
                    rhs=q_sb[koff0:koff0 + 32, kch0, ic * 512:(ic + 1) * 512],
                    start=True, stop=True, tile_position=(koff0, 0))
                nc.tensor.matmul(
                    dps[:, s, 512:1024],
                    lhsT=k_sb[koff1:koff1 + 32, kch1, jc * 128:(jc + 1) * 128],
                    rhs=q_sb[koff1:koff1 + 32, kch1, ic * 512:(ic + 1) * 512],
                    start=True, stop=True, tile_position=(koff1, 0))
                return s

            def emit_exp(p, ic, jc, s, attn, half):
                # 1024-wide exp: ring slice s -> attn[:, half] (2 heads x 512)
                nc.scalar.activation(
                    attn[:, half], dps[:, s, :].rearrange(
                        "p (h i) -> p h i", h=2), Exp)

            def emit_win(p, ic, jc, attn):
                # window multiply for both units / both heads in one DVE op
                base = 896 - 128 * jc + ic * 512
                w0 = win[:, 2 * p, base:base + 512]
                wview = AP(tensor=w0.tensor, offset=w0.offset,
                           ap=[list(w0.ap[0]), [-128, 2], [WIN, 2], [1, 512]])
                nc.vector.tensor_mul(attn, attn, wview)

            def emit_attnv(p, ic, jc, attn):
                for d in range(2):      # unit (jc+d)
                    for hl in range(2):  # head-local
                        nc.tensor.matmul(
                            aps[:, hl, :],
                            lhsT=v_aug[:, jc + d, 2 * p + hl, :],
                            rhs=attn[:, d, hl, :],
                            start=(jc + d == 0), stop=(jc + d == 7))

            def emit_norms(p, ic):
                # g[dv, i] = out_u[dv, i] / sums[i]
                for hl in range(2):
                    h = 2 * p + hl
                    bc = normp.tile([DV, 512], f32, tag="bc",
                                    name=f"bc_{h}_{ic}")
                    nc.vector.reciprocal_approx_fast(bc[0:32, :],
                                                     aps[0:32, hl, :])
                    nc.sync.dma_start(out=bc[32:DV, :], in_=bc[0:32, :])
                    nc.vector.tensor_mul(
                        g_sb[hl * DV:hl * DV + DV, p,
                             ic * 512:(ic + 1) * 512],
                        aps[DV:128, hl, :], bc)

            # unit stream.  fill work emitted between dots units to use PE
            # slack without blocking the ACT exp cadence.
            fill = [("vp", 0), ("vp", 2), ("vp", 4), ("vp", 6)]

            def emit_fill(n):
                for _ in range(n):
                    if not fill:
                        return
                    kind, a = fill.pop(0)
                    if kind == "vp":
                        emit_vproj2(a)

            # mc1 q/k projections are only needed by pairs 2/3; they reuse the
            # aps banks, so they may only run in the WAR window between a
            # pass's norms (last aps read) and the next pass's first attn@v
            # (next aps write).  Evacuation on DVE to keep ACT exp-pure.
            def emit_qkproj_boundary(ic):
                for (w_t, b_r, dst, slot) in ((wq, bqr, q_sb, 0),
                                              (wk, bkr, k_sb, 1)):
                    ps = aps[:, slot, :]
                    for kc in range(2):
                        nc.tensor.matmul(
                            ps,
                            lhsT=w_t[:, kc, 128:256],
                            rhs=xb[:, kc, ic * 512:(ic + 1) * 512],
                            start=(kc == 0), stop=False)
                    nc.tensor.matmul(
                        ps, lhsT=b_r[:, 128:256],
                        rhs=ones1[:, 0:512], start=False, stop=True)
                    nc.vector.tensor_copy(
                        dst[:, 1, ic * 512:(ic + 1) * 512], ps)

            attnv_q = []     # completed attn tiles awaiting attn@v

            for p in range(4):
                for ic in range(2):
                    for jc in (0, 2, 4, 6):
                        sa = emit_dots(p, ic, jc)
                        sb = emit_dots(p, ic, jc + 1)
                        attn = attnp.tile([128, 2, 2, 512], bf16, tag="attn",
                                          name=f"attn_{p}_{ic}_{jc}")
                        emit_exp(p, ic, jc, sa, attn, 0)
                        emit_exp(p, ic, jc + 1, sb, attn, 1)
                        emit_win(p, ic, jc, attn)
                        attnv_q.append((p, ic, jc, attn))
                        if len(attnv_q) > 1:
                            emit_attnv(*attnv_q.pop(0))
                        emit_fill(1)
                    # drain this pass's attn@v before norms
                    while attnv_q:
                        emit_attnv(*attnv_q.pop(0))
                    emit_norms(p, ic)
                    if p == 0:
                        emit_qkproj_boundary(ic)

            # ---- tail: gelus (one table-set load), out-proj, bias, store ----
            for kc in range(4):
                g_kc = g_sb[:, kc, :]
                nc.scalar.activation(g_kc, g_kc, Gelu)

            out_r = out_ext[:].rearrange("(a p) n -> p a n", p=128)
            fps = {}
            for ic in range(2):
                s = take_slice()
                for mc in range(2):
                    fps[(mc, ic)] = dps[:, s, mc * 512:(mc + 1) * 512]
            for kc in range(4):
                for ic in range(2):
                    for mc in range(2):
                        nc.tensor.matmul(
                            fps[(mc, ic)],
                            lhsT=wo[:, kc, mc * 128:(mc + 1) * 128],
                            rhs=g_sb[:, kc, ic * 512:(ic + 1) * 512],
                            start=(kc == 0), stop=(kc == 3))
            for ic in range(2):
                for mc in range(2):
                    o_sb = outp.tile([128, 512], f32, tag="osb",
                                     name=f"osb_{mc}_{ic}")
                    nc.vector.tensor_scalar_add(o_sb, fps[(mc, ic)],
                                                bos[:, mc:mc + 1])
                    nc.sync.dma_start(out=out_r[:, mc, ic * 512:(ic + 1) * 512],
                                      in_=o_sb)

    nc.finalize()
    return nc


_NC_CACHE = None


def kernel(**inputs) -> np.ndarray:
    global _NC_CACHE
    from concourse.bass_utils import run_bass_kernel_spmd

    x = np.asarray(inputs["x"], dtype=np.float32)
    shared = _prep_host(**inputs)

    if _NC_CACHE is None:
        _NC_CACHE = _build_nc()
    nc = _NC_CACHE

    in_maps = [dict(x=_x_shard(x, i), **shared) for i in range(B)]
    res = run_bass_kernel_spmd(nc, in_maps, core_ids=list(range(B)))
    out = np.stack([res.results[i]["out"] for i in range(B)], axis=0)
    return out.astype(np.float32)


# revision 21
# speedup vs baseline: 1.0519x; 1.0519x over previous
"""Multi-head attention (BN-folded QKV + rel-pos bias + GELU + out-proj) on 8 TRN2 cores.

Data-parallel over batch (b=8 -> 1 batch element per core, no collectives).

All BatchNorms are eval-mode affine transforms folded into the projection
weights/biases on the host.  The additive Toeplitz position bias is folded in
multiplicatively after exp:  exp(dots + E) = exp(dots) * exp(E), with exp(E)
shipped as per-partition pre-shifted window tiles.

v2 schedule (vs v1):
 - ACT (scalar) engine is the serial floor (64 softmax exps).  Exps are
   batched 2048-wide (one ACTIVATE per two attention units) reading a
   persistent 3-slice PSUM ring via strided APs.
 - ic is the outer loop within each head pair so the attn@v accumulators
   only hold 2 PSUM banks, freeing 6 banks for the dots ring.
 - v-projection units ride the same PSUM ring, interleaved with early
   attention units; q/k projections use the attnv accumulator banks before
   attention starts.
 - Input DMA is issued in consumption order on one queue (x, wq, wk, biases,
   wv, then the exp-window table in per-pair chunks) and dummy PE matmuls
   bridge the DMA wait so the PE hits the first projection with a warm HAM
   clock.
"""

import numpy as np
import ml_dtypes

HEADS, DK, DV = 8, 32, 64
DIM, N, DIM_OUT = 256, 1024, 256
IDK, IDV = DK * HEADS, DV * HEADS  # 256, 512
SCALE = DK ** -0.5
EPS = 1e-5
B = 8
WIN = 1920  # window tile width: covers i - 128*jc in [-896, 1023]

BF16 = ml_dtypes.bfloat16


def _prep_host(x, Wq, Wk, Wv, Wo, bo, pos_emb,
               q_gamma, q_beta, q_mean, q_var,
               k_gamma, k_beta, k_mean, k_var,
               v_gamma, v_beta, v_mean, v_var,
               o_gamma, o_beta, o_mean, o_var):
    f32 = np.float32
    inv_q = (q_gamma / np.sqrt(q_var + EPS)).astype(f32)
    inv_k = (k_gamma / np.sqrt(k_var + EPS)).astype(f32)
    inv_v = (v_gamma / np.sqrt(v_var + EPS)).astype(f32)
    inv_o = (o_gamma / np.sqrt(o_var + EPS)).astype(f32)

    # q also absorbs the attention scale
    Wq_eff = (Wq * inv_q[:, None]) * SCALE
    bq = ((q_beta - q_mean * inv_q) * SCALE).astype(f32)
    Wk_eff = Wk * inv_k[:, None]
    bk = (k_beta - k_mean * inv_k).astype(f32)
    Wv_eff = Wv * inv_v[:, None]
    bv = (v_beta - v_mean * inv_v).astype(f32)
    Wo_eff = Wo * inv_o[:, None]
    bo_eff = ((bo - o_mean) * inv_o + o_beta).astype(f32)

    # lhsT layouts, pre-chunked to the exact SBUF tile shapes
    def chunk_T(w, kchunks):  # [O, C] -> [128, kchunks, O]  (WT[c, o] tiled)
        wT = np.ascontiguousarray(w.T.astype(f32))  # [C, O]
        c, o = wT.shape
        assert c == kchunks * 128
        return np.ascontiguousarray(
            wT.reshape(kchunks, 128, o).transpose(1, 0, 2)).astype(BF16)

    wqs = chunk_T(Wq_eff, 2)            # [128, 2, 256]
    wks = chunk_T(Wk_eff, 2)            # [128, 2, 256]
    wvs = chunk_T(Wv_eff, 2)            # [128, 2, 512]
    wos = chunk_T(Wo_eff, 4)            # [128, 4, 256]

    def chunk_bias(b, chunks):  # [C] -> [128, chunks]
        return np.ascontiguousarray(
            b.reshape(chunks, 128).T).astype(f32)

    bos = chunk_bias(bo_eff, 2)         # [128, 2]
    # q/k/v biases enter their projections as a ones-row matmul (K=1)
    bqrow = bq.reshape(1, IDK).astype(BF16)  # [1, 256]
    bkrow = bk.reshape(1, IDK).astype(BF16)  # [1, 256]
    bvrow = bv.reshape(1, IDV).astype(BF16)  # [1, 512]  (h-major: c = h*64+dv)

    # exp-window tiles: for j = 128*jc + p, attn[p, i] needs
    # expE[1023 + i - j] = win[p, h, (896 - 128*jc) + i]
    E = (np.asarray(pos_emb, dtype=np.float64) / SCALE)  # [N, HEADS]
    d = np.abs(np.arange(2047) - 1023)
    expE = np.exp(E[d, :])  # [2047, HEADS] float64
    idx = np.arange(WIN)[None, :] - np.arange(128)[:, None] + 127  # [128, WIN]
    win = expE[idx, :].transpose(0, 2, 1)  # [128, HEADS, WIN]
    win = np.ascontiguousarray(win).astype(BF16)

    shared = dict(wqs=wqs, wks=wks, wvs=wvs, wos=wos,
                  bqrow=bqrow, bkrow=bkrow, bos=bos, bvrow=bvrow, win=win)
    return shared


def _x_shard(x, i):
    # device consumes x as bf16 [128, 2, n] (channel chunks c = a*128 + p)
    xi = np.asarray(x[i], dtype=np.float32).reshape(2, 128, N).transpose(1, 0, 2)
    return np.ascontiguousarray(xi).astype(BF16)


def _build_nc():
    import concourse.bass as bass
    import concourse.mybir as mybir
    import concourse.tile as tile
    from concourse import bacc

    f32 = mybir.dt.float32
    bf16 = mybir.dt.bfloat16

    nc = bacc.Bacc(None, target_bir_lowering=False)

    x_ext = nc.declare_dram_parameter("x", [128, 2, N], bf16, isOutput=False)
    wqs_ext = nc.declare_dram_parameter("wqs", [128, 2, IDK], bf16, isOutput=False)
    wks_ext = nc.declare_dram_parameter("wks", [128, 2, IDK], bf16, isOutput=False)
    wvs_ext = nc.declare_dram_parameter("wvs", [128, 2, IDV], bf16, isOutput=False)
    wos_ext = nc.declare_dram_parameter("wos", [128, 4, DIM_OUT], bf16, isOutput=False)
    bqrow_ext = nc.declare_dram_parameter("bqrow", [1, IDK], bf16, isOutput=False)
    bkrow_ext = nc.declare_dram_parameter("bkrow", [1, IDK], bf16, isOutput=False)
    bos_ext = nc.declare_dram_parameter("bos", [128, 2], f32, isOutput=False)
    bvrow_ext = nc.declare_dram_parameter("bvrow", [1, IDV], bf16, isOutput=False)
    win_ext = nc.declare_dram_parameter("win", [128, HEADS, WIN], bf16, isOutput=False)
    out_ext = nc.declare_dram_parameter("out", [DIM_OUT, N], f32, isOutput=True)

    Exp = mybir.ActivationFunctionType.Exp
    Gelu = mybir.ActivationFunctionType.Gelu
    Identity = mybir.ActivationFunctionType.Identity

    AP = bass.AP

    with tile.TileContext(nc) as tc:
        with (
            tc.tile_pool(name="consts", bufs=1) as consts,
            tc.tile_pool(name="attnp", bufs=5) as attnp,
            tc.tile_pool(name="normp", bufs=4) as normp,
            tc.tile_pool(name="outp", bufs=4) as outp,
            tc.tile_pool(name="psum", bufs=1, space="PSUM") as psum,
        ):
            # ---- persistent PSUM: dots ring (6 banks) + attnv accum (2) ----
            dps = psum.tile([128, 3, 1024], f32)   # ring of 3 unit slices
            aps = psum.tile([128, 2, 512], f32)    # attnv accum / qkproj psums

            # ---- SBUF constants / intermediates ----
            xb = consts.tile([128, 2, N], bf16)
            wq = consts.tile([128, 2, IDK], bf16)
            wk = consts.tile([128, 2, IDK], bf16)
            wv = consts.tile([128, 2, IDV], bf16)
            wo = consts.tile([128, 4, DIM_OUT], bf16)
            bqr = consts.tile([1, IDK], bf16)
            bkr = consts.tile([1, IDK], bf16)
            bos = consts.tile([128, 2], f32)
            bvr = consts.tile([1, IDV], bf16)
            win = consts.tile([128, HEADS, WIN], bf16)
            ones1 = consts.tile([1, 512], bf16)

            q_sb = consts.tile([128, 2, N], bf16)   # [ (h,d) chunks, i ]
            k_sb = consts.tile([128, 2, N], bf16)   # [ (h,d) chunks, j ]
            # v columns 0:32 ones (sums 32-replicated for the DVE reciprocal),
            # 32:64 zero (dark PE cells; PSUM reads must stay quadrant-aligned
            # so out_u lives at partitions 64:128), 64:128 = v + bv.
            v_aug = consts.tile([128, 8, HEADS, 128], bf16)  # [j-part, jc, h, one|0|dv]
            g_sb = consts.tile([128, 4, N], bf16)   # gelu in/out [(h,dv) chunks, i]

            # ---- input DMA, single queue, consumption order ----
            nc.sync.dma_start(out=xb, in_=x_ext[:])
            nc.sync.dma_start(out=wq, in_=wqs_ext[:])
            nc.sync.dma_start(out=wk, in_=wks_ext[:])
            nc.sync.dma_start(out=bqr, in_=bqrow_ext[:])
            nc.sync.dma_start(out=bkr, in_=bkrow_ext[:])
            nc.sync.dma_start(out=wv, in_=wvs_ext[:])
            nc.sync.dma_start(out=bvr, in_=bvrow_ext[:])
            # win chunks: pair p needs heads 2p:2p+2; u>=768 for jc pair (0,1),
            # then 256-wide strips marching left for jc pairs (2,3),(4,5),(6,7)
            win_ranges = [(768, 1920), (512, 768), (256, 512), (0, 256)]
            win_done = {}
            for p in range(4):
                for ci, (u0, u1) in enumerate(win_ranges):
                    nc.sync.dma_start(
                        out=win[:, 2 * p:2 * p + 2, u0:u1],
                        in_=win_ext[:, 2 * p:2 * p + 2, u0:u1])
                    win_done[(p, ci)] = True
                if p == 0:
                    nc.sync.dma_start(out=wo, in_=wos_ext[:])
                    nc.sync.dma_start(out=bos, in_=bos_ext[:])

            # ---- engine warmups (no DMA deps) ----
            nc.vector.memset(ones1, 1.0)
            # ones/zero regions of v_aug on gpsimd (idle engine)
            nc.gpsimd.memset(v_aug[:, :, :, 0:32], 1.0)
            nc.gpsimd.memset(v_aug[:, :, :, 32:DV], 0.0)
            # ACT exp-table warm load
            warm = normp.tile([1, 8], f32, tag="warm", bufs=1)
            nc.vector.memset(warm, 1.0)
            nc.scalar.activation(warm, warm, Exp)
            # PE warm matmuls: keep the PE busy through the x/wq DMA wait so
            # HAM grants the 2.4GHz clock by the time real work lands.
            for _ in range(8):
                nc.tensor.matmul(aps[:, 0, :], lhsT=ones1[:, 0:128],
                                 rhs=ones1[:, 0:512], start=True, stop=True)

            # ---- q/k projections: psums on the aps ring.  The channel bias
            # rides a K=1 ones-row matmul so the evacuation is a plain
            # table-free copy on ACT (no Identity table-set thrash). ----
            def emit_qkproj(mc, w_t, b_r, dst, ic, slot):
                ps = aps[:, slot, :]
                for kc in range(2):
                    nc.tensor.matmul(
                        ps,
                        lhsT=w_t[:, kc, mc * 128:(mc + 1) * 128],
                        rhs=xb[:, kc, ic * 512:(ic + 1) * 512],
                        start=(kc == 0), stop=False)
                nc.tensor.matmul(
                    ps, lhsT=b_r[:, mc * 128:(mc + 1) * 128],
                    rhs=ones1[:, 0:512], start=False, stop=True)
                nc.scalar.copy(dst[:, mc, ic * 512:(ic + 1) * 512], ps)

            # mc0 q/k ic0+ic1 first: they gate pair-0 attention
            emit_qkproj(0, wq, bqr, q_sb, 0, 0)
            emit_qkproj(0, wk, bkr, k_sb, 0, 1)
            emit_qkproj(0, wq, bqr, q_sb, 1, 0)
            emit_qkproj(0, wk, bkr, k_sb, 1, 1)

            # ---- ring state ----
            ring = [0]  # next dps slice index

            def take_slice():
                s = ring[0]
                ring[0] = (s + 1) % 3
                return s

            # ---- v projection pseudo-units: two jc per ring slice ----
            def emit_vproj2(jc):  # jc, jc+1
                s = take_slice()
                for d in range(2):
                    ps = dps[:, s, d * 512:(d + 1) * 512]
                    for kc in range(2):
                        nc.tensor.matmul(
                            ps,
                            lhsT=xb[:, kc, (jc + d) * 128:(jc + d + 1) * 128],
                            rhs=wv[:, kc, :],
                            start=(kc == 0), stop=False)
                    nc.tensor.matmul(ps, lhsT=ones1[:, 0:128], rhs=bvr,
                                     start=False, stop=True)
                # one batched cast for both jc
                nc.vector.tensor_copy(
                    v_aug[:, jc:jc + 2, :, DV:128],
                    dps[:, s, :].rearrange("p (j h d) -> p j h d", j=2, h=HEADS))

            # ---- attention ----
            KOFF = [((h % 4) * 32, h // 4) for h in range(HEADS)]

            def emit_dots(p, ic, jc):
                s = take_slice()
                h0, h1 = 2 * p, 2 * p + 1
                koff0, kch0 = KOFF[h0]
                koff1, kch1 = KOFF[h1]
                nc.tensor.matmul(
                    dps[:, s, 0:512],
                    lhsT=k_sb[koff0:koff0 + 32, kch0, jc * 128:(jc + 1) * 128],
                    rhs=q_sb[koff0:koff0 + 32, kch0, ic * 512:(ic + 1) * 512],
                    start=True, stop=True, tile_position=(koff0, 0))
                nc.tensor.matmul(
                    dps[:, s, 512:1024],
                    lhsT=k_sb[koff1:koff1 + 32, kch1, jc * 128:(jc + 1) * 128],
                    rhs=q_sb[koff1:koff1 + 32, kch1, ic * 512:(ic + 1) * 512],
                    start=True, stop=True, tile_position=(koff1, 0))
                return s

            def emit_exp(p, ic, jc, s, attn, half):
                # 1024-wide exp: ring slice s -> attn[:, half] (2 heads x 512)
                nc.scalar.activation(
                    attn[:, half], dps[:, s, :].rearrange(
                        "p (h i) -> p h i", h=2), Exp)

            def emit_win(p, ic, jc, attn):
                # window multiply for both units / both heads in one DVE op
                base = 896 - 128 * jc + ic * 512
                w0 = win[:, 2 * p, base:base + 512]
                wview = AP(tensor=w0.tensor, offset=w0.offset,
                           ap=[list(w0.ap[0]), [-128, 2], [WIN, 2], [1, 512]])
                nc.vector.tensor_mul(attn, attn, wview)

            def emit_attnv(p, ic, jc, attn):
                for d in range(2):      # unit (jc+d)
                    for hl in range(2):  # head-local
                        nc.tensor.matmul(
                            aps[:, hl, :],
                            lhsT=v_aug[:, jc + d, 2 * p + hl, :],
                            rhs=attn[:, d, hl, :],
                            start=(jc + d == 0), stop=(jc + d == 7))

            def emit_norms(p, ic):
                # g[dv, i] = out_u[dv, i] / sums[i]
                for hl in range(2):
                    h = 2 * p + hl
                    bc = normp.tile([DV, 512], f32, tag="bc",
                                    name=f"bc_{h}_{ic}")
                    nc.vector.reciprocal_approx_fast(bc[0:32, :],
                                                     aps[0:32, hl, :])
                    nc.sync.dma_start(out=bc[32:DV, :], in_=bc[0:32, :])
                    nc.vector.tensor_mul(
                        g_sb[hl * DV:hl * DV + DV, p,
                             ic * 512:(ic + 1) * 512],
                        aps[DV:128, hl, :], bc)

            # unit stream.  fill work emitted between dots units to use PE
            # slack without blocking the ACT exp cadence.
            fill = [("vp", 0), ("vp", 2), ("vp", 4), ("vp", 6)]

            def emit_fill(n):
                for _ in range(n):
                    if not fill:
                        return
                    kind, a = fill.pop(0)
                    if kind == "vp":
                        emit_vproj2(a)

            # mc1 q/k projections are only needed by pairs 2/3; they reuse the
            # aps banks, so they may only run in the WAR window between a
            # pass's norms (last aps read) and the next pass's first attn@v
            # (next aps write).  Evacuation on DVE to keep ACT exp-pure.
            def emit_qkproj_boundary(ic):
                for (w_t, b_r, dst, slot) in ((wq, bqr, q_sb, 0),
                                              (wk, bkr, k_sb, 1)):
                    ps = aps[:, slot, :]
                    for kc in range(2):
                        nc.tensor.matmul(
                            ps,
                            lhsT=w_t[:, kc, 128:256],
                            rhs=xb[:, kc, ic * 512:(ic + 1) * 512],
                            start=(kc == 0), stop=False)
                    nc.tensor.matmul(
                        ps, lhsT=b_r[:, 128:256],
                        rhs=ones1[:, 0:512], start=False, stop=True)
                    nc.vector.tensor_copy(
                        dst[:, 1, ic * 512:(ic + 1) * 512], ps)

            attnv_q = []     # completed attn tiles awaiting attn@v

            def pop_attnv():
                pp, pic, pjc, pattn = attnv_q.pop(0)
                emit_attnv(pp, pic, pjc, pattn)
                if pjc == 6:
                    # pass (pp, pic) fully accumulated: normalize, and run
                    # the mc1 q/k projections in the aps WAR window
                    emit_norms(pp, pic)
                    if pp == 0:
                        emit_qkproj_boundary(pic)

            # attn@v lags TWO pairs so its winmul dependency is long-resolved
            # when the PE (in-order queue) reaches it -- otherwise the
            # exp->win->attnv->dots chain paces the whole pipeline.
            for p in range(4):
                for ic in range(2):
                    for jc in (0, 2, 4, 6):
                        sa = emit_dots(p, ic, jc)
                        sb = emit_dots(p, ic, jc + 1)
                        attn = attnp.tile([128, 2, 2, 512], bf16, tag="attn",
                                          name=f"attn_{p}_{ic}_{jc}")
                        emit_exp(p, ic, jc, sa, attn, 0)
                        emit_exp(p, ic, jc + 1, sb, attn, 1)
                        emit_win(p, ic, jc, attn)
                        attnv_q.append((p, ic, jc, attn))
                        if len(attnv_q) > 2:
                            pop_attnv()
                        emit_fill(1)
            while attnv_q:
                pop_attnv()

            # ---- tail: gelus (one table-set load), out-proj, bias, store ----
            for kc in range(4):
                g_kc = g_sb[:, kc, :]
                nc.scalar.activation(g_kc, g_kc, Gelu)

            out_r = out_ext[:].rearrange("(a p) n -> p a n", p=128)
            fps = {}
            for ic in range(2):
                s = take_slice()
                for mc in range(2):
                    fps[(mc, ic)] = dps[:, s, mc * 512:(mc + 1) * 512]
            for kc in range(4):
                for ic in range(2):
                    for mc in range(2):
                        nc.tensor.matmul(
                            fps[(mc, ic)],
                            lhsT=wo[:, kc, mc * 128:(mc + 1) * 128],
                            rhs=g_sb[:, kc, ic * 512:(ic + 1) * 512],
                            start=(kc == 0), stop=(kc == 3))
            for ic in range(2):
                for mc in range(2):
                    o_sb = outp.tile([128, 512], f32, tag="osb",
                                     name=f"osb_{mc}_{ic}")
                    nc.vector.tensor_scalar_add(o_sb, fps[(mc, ic)],
                                                bos[:, mc:mc + 1])
                    nc.sync.dma_start(out=out_r[:, mc, ic * 512:(ic + 1) * 512],
                                      in_=o_sb)

    nc.finalize()
    return nc


_NC_CACHE = None


def kernel(**inputs) -> np.ndarray:
    global _NC_CACHE
    from concourse.bass_utils import run_bass_kernel_spmd

    x = np.asarray(inputs["x"], dtype=np.float32)
    shared = _prep_host(**inputs)

    if _NC_CACHE is None:
        _NC_CACHE = _build_nc()
    nc = _NC_CACHE

    in_maps = [dict(x=_x_shard(x, i), **shared) for i in range(B)]
    res = run_bass_kernel_spmd(nc, in_maps, core_ids=list(range(B)))
    out = np.stack([res.results[i]["out"] for i in range(B)], axis=0)
    return out.astype(np.float32)
